# revision 26
# baseline (speedup 1.0000x reference)
"""Trainium2 Bass kernel for nn_AttnKnn (retrieval KNN attention), 8-core SPMD.

Sharding: core = (batch b, n-half h); each core runs the full pipeline for its
1024 query points. Only cross-core exchange: pairwise AllGather of x2 column
halves (dense stage is m-row-split; fc stage needs n-column-split).

Key structure vs the naive version:
- Neighbor gather uses batched SWDGE dma_gather (1024 rows per instruction,
  ring-limited) instead of per-k indirect DMAs: kvx rows are [V | x] (512B).
- K projection is folded into the query side: logits = sum_c xwn_c * tq_c
  with tq = (x^T Wq) Wk^T, so K rows are never gathered.
- The wrapped int16 index layout dma_gather needs ([16, n/16] replicated) is
  built on-chip with 8 partition-shuffle matmuls against a constant matrix.
- Software pipeline: the PE-bound dense stage (P2) overlaps the DVE-bound
  KNN stage (P1); the fc stage (P3a) overlaps the final gather (P3b).

Assumes (asserted on host): g1 == g2 == 1, beta1 == beta2 == 0, biases == 0
(guaranteed by the problem spec fills).
"""
import numpy as np

B, C, N, Q = 4, 64, 2048, 64
KT, K = 50, 20
NL = N // 2
ML = NL * KT * C // N          # 1600
NBLK = NL // 128               # 8
SCALE = float(1.0 / np.sqrt(np.float32(N)))
LN_EPS = 1e-5
NEG = -1e30


def build_nc(debug=False, nocc=False):
    import concourse.bass as bass
    import concourse.bacc as bacc
    import concourse.mybir as mybir
    from concourse.tile import TileContext
    from concourse import library_config

    f32 = mybir.dt.float32
    f32r = mybir.dt.float32r
    bf16 = mybir.dt.bfloat16
    i16 = mybir.dt.int16
    u16 = mybir.dt.uint16
    ACTF = mybir.ActivationFunctionType
    OP = mybir.AluOpType
    AX = mybir.AxisListType
    AP = bass.AP

    def bcast_inner(t, inner):
        return AP(t.tensor, t.offset, [list(d) for d in t.ap] + [[0, inner]])

    nc = bacc.Bacc("TRN2", target_bir_lowering=False, num_devices=8)

    xb_d = nc.dram_tensor("xb", [C, N], f32, kind="ExternalInput")
    xq_d = nc.dram_tensor("xq", [C, NL], f32, kind="ExternalInput")
    wq_d = nc.dram_tensor("wq", [C, Q], f32, kind="ExternalInput")
    wk_d = nc.dram_tensor("wk", [C, Q], f32, kind="ExternalInput")
    wv_d = nc.dram_tensor("wv", [C, C], f32, kind="ExternalInput")
    wd_d = nc.dram_tensor("wd", [N, N], f32, kind="ExternalInput")
    wfc_d = nc.dram_tensor("wfc", [C, C], f32, kind="ExternalInput")
    ident_d = nc.dram_tensor("ident", [128, 128], f32, kind="ExternalInput")
    pcol_d = nc.dram_tensor("pcol", [128, 1], f32, kind="ExternalInput")  # 50*p
    shuf_d = nc.dram_tensor("shuf", [128, 1024], f32, kind="ExternalInput")
    out_d = nc.dram_tensor("outp", [C, K, NL], f32, kind="ExternalOutput")

    kvx_d = nc.dram_tensor("kvx", [N, 128], f32)
    z_d = nc.dram_tensor("zs", [NL * KT * C], f32)
    xwt_d = nc.dram_tensor("xwt", [KT * C, NL], f32)
    x2s_d = nc.dram_tensor("x2s", [ML * N], f32)
    xg_d = nc.dram_tensor("xg", [2, ML * N], f32)
    x2a_d = nc.dram_tensor("x2a", [2 * ML, NL], f32)
    x5_d = nc.dram_tensor("x5d", [NL * KT, C], f32)
    if debug:
        dbg_idx = nc.dram_tensor("dbg_idx", [NL, KT], u16, kind="ExternalOutput")
        dbg_widx = nc.dram_tensor("dbg_widx", [128, 400], u16, kind="ExternalOutput")
        dbg_z = nc.dram_tensor("dbg_z", [NL * KT * C], f32, kind="ExternalOutput")
        dbg_x2a = nc.dram_tensor("dbg_x2a", [2 * ML, NL], f32, kind="ExternalOutput")

    with TileContext(nc) as tc:
        nc.gpsimd.load_library(library_config.mlp)
        r1k = nc.gpsimd.to_reg(1024)
        r512 = nc.gpsimd.to_reg(512)
        r256 = nc.gpsimd.to_reg(256)
        persist = tc.tile_pool(name="persist", bufs=1)
        pp = persist.__enter__()
        ident = pp.tile([128, 128], f32)
        nc.sync.dma_start(ident, ident_d[:, :])
        pcol = pp.tile([128, 1], f32)
        nc.sync.dma_start(pcol, pcol_d[:, :])
        shufs = pp.tile([128, 1024], f32)
        nc.sync.dma_start(shufs, shuf_d[:, :])
        wfcs = pp.tile([C, C], f32)
        nc.sync.dma_start(wfcs, wfc_d[:, :])
        bd0 = pp.tile([128, 128], f32)
        nc.vector.memset(bd0, 0.0)
        nc.vector.tensor_copy(bd0[0:64, 0:64], wfcs)
        nc.vector.tensor_copy(bd0[64:128, 64:128], wfcs)
        bdr = pp.tile([128, 128], f32r)
        nc.vector.tensor_copy(bdr, bd0)
        msqn = pp.tile([128, NBLK], f32)
        tqa = pp.tile([128, NBLK * C], f32)
        xbar = pp.tile([65, N], f32)
        xq2r = pp.tile([65, NL], f32)
        widx5 = pp.tile([128, NBLK * 8 * K], u16)   # wrapped fc-gather indices

        # ---------------- P0 ----------------
        with tc.tile_pool(name="p0", bufs=2) as p0, \
             tc.tile_pool(name="p0ps", bufs=2, space="PSUM") as p0ps:
            xbs = p0.tile([C, N], f32, bufs=1)
            nc.sync.dma_start(xbs, xb_d[:, :])
            xqs = p0.tile([C, NL], f32, bufs=1)
            nc.sync.dma_start(xqs, xq_d[:, :])
            wqs = p0.tile([C, Q], f32, bufs=1)
            nc.sync.dma_start(wqs, wq_d[:, :])
            wks = p0.tile([C, Q], f32, bufs=1)
            nc.sync.dma_start(wks, wk_d[:, :])
            wvs = p0.tile([C, C], f32, bufs=1)
            nc.sync.dma_start(wvs, wv_d[:, :])
            ones = p0.tile([C, 1], f32, bufs=1)
            nc.vector.memset(ones, 1.0)
            Ms = p0.tile([C, C], f32, bufs=1)
            sqrow = p0.tile([1, N], f32, bufs=1)
            sqqrow = p0.tile([1, NL], f32, bufs=1)
            xba = p0.tile([65, N], f32, bufs=1)
            xq2 = p0.tile([65, NL], f32, bufs=1)
            # M = Wq @ Wk^T  (for logits = sum_c xwn_c * (q @ Wk^T)_c)
            pwq = p0ps.tile([Q, C], f32, tag="ps0")
            nc.tensor.transpose(pwq, wqs, ident[0:C, 0:C])
            wqT = p0.tile([Q, C], f32, bufs=1)
            nc.scalar.copy(wqT, pwq)
            pwk = p0ps.tile([Q, C], f32, tag="ps0")
            nc.tensor.transpose(pwk, wks, ident[0:C, 0:C])
            wkT = p0.tile([Q, C], f32, bufs=1)
            nc.scalar.copy(wkT, pwk)
            pM = p0ps.tile([C, C], f32, tag="ps0")
            nc.tensor.matmul(pM, wqT, wkT, start=True, stop=True)
            nc.scalar.copy(Ms, pM)

            xsq = p0.tile([C, N], f32, bufs=1)
            nc.scalar.square(xsq, xbs)
            for j in range(4):
                pq = p0ps.tile([1, 512], f32, tag="ps0")
                nc.tensor.matmul(pq, ones, xsq[:, 512 * j:512 * (j + 1)],
                                 start=True, stop=True)
                nc.scalar.copy(sqrow[:, 512 * j:512 * (j + 1)], pq)
            xsqq = p0.tile([C, NL], f32, bufs=1)
            nc.scalar.square(xsqq, xqs)
            for j in range(2):
                pq2 = p0ps.tile([1, 512], f32, tag="ps0")
                nc.tensor.matmul(pq2, ones, xsqq[:, 512 * j:512 * (j + 1)],
                                 start=True, stop=True)
                nc.scalar.copy(sqqrow[:, 512 * j:512 * (j + 1)], pq2)
            # kvx rows: [V | x^T]
            for j in range(16):
                sl = slice(128 * j, 128 * (j + 1))
                pt = p0ps.tile([128, C], f32, tag="ps0")
                nc.tensor.transpose(pt, xbs[:, sl], ident[0:C, 0:C])
                xtt = p0.tile([128, C], f32)
                nc.scalar.copy(xtt, pt)
                nc.sync.dma_start(kvx_d[sl, 64:128], xtt)
                pv = p0ps.tile([128, C], f32, tag="ps0")
                nc.tensor.matmul(pv, xbs[:, sl], wvs, start=True, stop=True)
                vat = p0.tile([128, C], f32)
                nc.scalar.copy(vat, pv)
                nc.sync.dma_start(kvx_d[sl, 0:64], vat)
            nc.vector.tensor_copy(xba[0:C, :], xbs)
            nc.scalar.activation(xba[C:C + 1, :], sqrow, ACTF.Identity,
                                 bias=0.0, scale=-1.0)
            nc.scalar.activation(xq2[0:C, :], xqs, ACTF.Identity,
                                 bias=0.0, scale=2.0)
            nc.vector.memset(xq2[C:C + 1, :], 1.0)
            nc.vector.tensor_copy(xbar, xba)
            nc.vector.tensor_copy(xq2r, xq2)
            for blk in range(NBLK):
                sl = slice(128 * blk, 128 * (blk + 1))
                ptq = p0ps.tile([128, C], f32, tag="ps0")
                nc.tensor.matmul(ptq, xqs[:, sl], Ms, start=True, stop=True)
                nc.scalar.copy(tqa[:, C * blk:C * (blk + 1)], ptq)
                psn = p0ps.tile([128, 1], f32, tag="ps0")
                nc.tensor.transpose(psn, sqqrow[:, sl], ident[0:1, 0:1])
                nc.scalar.activation(msqn[:, blk:blk + 1], psn, ACTF.Identity,
                                     bias=0.0, scale=-1.0)

        tc.strict_bb_all_engine_barrier()

        # ---------------- P1 + P2 software pipeline ----------------
        with tc.tile_pool(name="p1", bufs=2) as p1, \
             tc.tile_pool(name="pps", bufs=2, space="PSUM") as pps, \
             tc.tile_pool(name="wd", bufs=1) as wdp, \
             tc.tile_pool(name="p2", bufs=2) as p2:

            def p1_stageA(blk):
                nsl = slice(128 * blk, 128 * (blk + 1))
                dist = p1.tile([128, N], f32, tag="dist")
                for j in range(4):
                    pd = pps.tile([128, 512], f32, tag="pd", bufs=1)
                    nc.tensor.matmul(pd, xq2r[:, nsl], xbar[:, 512 * j:512 * (j + 1)],
                                     start=True, stop=True)
                    nc.scalar.activation(dist[:, 512 * j:512 * (j + 1)], pd,
                                         ACTF.Identity, bias=msqn[:, blk:blk + 1],
                                         scale=1.0)
                ti = p1.tile([128, 56], u16, tag="ti")
                m8 = p1.tile([128, 8], f32, tag="m8")
                for it in range(7):
                    nc.vector.max(out=m8, in_=dist)
                    nc.vector.max_index(out=ti[:, 8 * it:8 * (it + 1)], in_max=m8,
                                        in_values=dist)
                    if it < 6:
                        nc.vector.match_replace(out=dist, in_to_replace=m8,
                                                in_values=dist, imm_value=NEG)
                if debug:
                    nc.sync.dma_start(dbg_idx[nsl, :], ti[:, 0:KT])
                tif = p1.tile([128, KT], f32, tag="tif")
                nc.vector.tensor_copy(tif, ti[:, 0:KT])
                return tif

            def p1_stageG(blk, tif):
                # wrapped idx layout: widx[p, 8i+b] = ti[16b+p%16, i]
                widx = p1.tile([128, 8 * KT], u16, tag="widx")
                for b in range(8):
                    psh = pps.tile([128, KT], f32, tag="sh", bufs=1)
                    nc.tensor.matmul(psh, shufs[:, 128 * b:128 * (b + 1)], tif,
                                     start=True, stop=True)
                    nc.scalar.copy(
                        AP(widx.tensor, widx.offset + b,
                           [list(widx.ap[0]), [8, KT]]), psh)
                if debug:
                    nc.sync.dma_start(dbg_widx[:, :], widx)
                # batched neighbor gather: rows [V | x], 512B each
                gbuf = p1.tile([128, KT, 128], f32, tag="gbuf", bufs=1)
                for j in range(7):
                    nidx = 1024 if j < 6 else 256
                    ks = slice(8 * j, 8 * j + nidx // 128)
                    idx = widx[:, 64 * j:64 * j + nidx // 16].bitcast(i16)
                    reg = r1k if nidx == 1024 else r256
                    nc.gpsimd.dma_gather(gbuf[:, ks, :], kvx_d[:, :], idx,
                                         nidx, reg, 128)
                return (gbuf,)

            def p1_stageB(blk, gbuf):
                nsl = slice(128 * blk, 128 * (blk + 1))
                vbuf = gbuf[:, :, 0:64]
                xbuf = gbuf[:, :, 64:128]
                # xwn^T chunks -> DRAM (transpose straight out of gbuf);
                # emitted first so these Act copies drain before the softmax.
                # BIR allows only one free dim on transpose inputs, so
                # transpose per-k [128,64] slices and pair-pack the results.
                for c2 in range(25):
                    xwtt = p1.tile([128, 128], f32, tag="xwtt")
                    for h2 in range(2):
                        pxw = pps.tile([64, 128], f32, tag="pxw")
                        nc.tensor.transpose(pxw, gbuf[:, 2 * c2 + h2, 64:128],
                                            ident)
                        nc.scalar.copy(xwtt[64 * h2:64 * (h2 + 1), :], pxw)
                    nc.sync.dma_start(xwt_d[128 * c2:128 * (c2 + 1), nsl], xwtt)
                # logits = sum_c x_c * tq_c
                qsl = tqa[:, C * blk:C * (blk + 1)]
                prod = p1.tile([128, KT, Q], f32, tag="pz")
                nc.vector.tensor_mul(prod, xbuf,
                                     AP(qsl.tensor, qsl.offset,
                                        [list(qsl.ap[0]), [0, KT], list(qsl.ap[1])]))
                logits = p1.tile([128, KT], f32, tag="logits")
                nc.vector.tensor_reduce(out=logits, in_=prod, op=OP.add, axis=AX.X)
                lm8 = p1.tile([128, 8], f32, tag="lm8")
                nc.vector.max(out=lm8, in_=logits)
                mneg = p1.tile([128, 1], f32, tag="mneg")
                nc.vector.tensor_scalar_mul(mneg, lm8[:, 0:1], -SCALE)
                attnu = p1.tile([128, KT], f32, tag="attnu")
                nc.scalar.activation(attnu, logits, ACTF.Exp, bias=mneg, scale=SCALE)
                asum = p1.tile([128, 1], f32, tag="asum")
                nc.vector.tensor_reduce(out=asum, in_=attnu, op=OP.add, axis=AX.X)
                rsum = p1.tile([128, 1], f32, tag="rsum")
                nc.vector.reciprocal(rsum, asum)
                attn = p1.tile([128, KT], f32, tag="attn")
                nc.vector.tensor_scalar_mul(attn, attnu, rsum)
                # bottom-20 of logits (== argsort(attn)[:20])
                negl = p1.tile([128, KT], f32, tag="negl")
                nc.vector.tensor_scalar_mul(negl, logits, -1.0)
                kti = p1.tile([128, 24], u16, tag="kti")
                nm8 = p1.tile([128, 8], f32, tag="nm8")
                for it in range(3):
                    nc.vector.max(out=nm8, in_=negl)
                    nc.vector.max_index(out=kti[:, 8 * it:8 * (it + 1)], in_max=nm8,
                                        in_values=negl)
                    if it < 2:
                        nc.vector.match_replace(out=negl, in_to_replace=nm8,
                                                in_values=negl, imm_value=NEG)
                # z = attn*v + xwn -> DRAM
                z3 = p1.tile([128, KT, C], f32, tag="pz")
                nc.vector.tensor_mul(z3, vbuf, bcast_inner(attn, C))
                nc.vector.tensor_add(z3, z3, xbuf)
                zview = z_d.ap()[blk * 409600:(blk + 1) * 409600].rearrange(
                    "(p f) -> p f", p=128)
                nc.sync.dma_start(zview, z3)
                # fc-gather indices (x5 row = 50*q_local + k), wrapped per block
                kself = p1.tile([128, 24], f32, tag="kself")
                nc.vector.tensor_copy(kself, kti)
                kofff = p1.tile([128, K], f32, tag="kofff")
                nc.vector.tensor_add(kofff, kself[:, 0:K],
                                     AP(pcol.tensor, pcol.offset,
                                        [list(pcol.ap[0]), [0, K]]))
                for b in range(8):
                    psh5 = pps.tile([128, KT], f32, tag="sh", bufs=1)
                    nc.tensor.matmul(psh5[:, 0:K], shufs[:, 128 * b:128 * (b + 1)], kofff,
                                     start=True, stop=True)
                    nc.scalar.copy(
                        AP(widx5.tensor, widx5.offset + 8 * K * blk + b,
                           [list(widx5.ap[0]), [8, K]]), psh5[:, 0:K])

            wdb = []

            def ln_scales(pool, pref, src, rows, width, scratch):
                """mean/var over `width` via two Act accumulate passes writing
                throwaway data into `scratch` (a dead/overwritten-later tile);
                returns (rstd, mrn) with mrn = -mean*rstd, both [rows, 1]."""
                s1 = pool.tile([128, 1], f32, tag=f"{pref}s1")
                nc.scalar.activation(scratch[0:rows, 0:width], src, ACTF.Identity,
                                     accum_out=s1[0:rows])
                s2 = pool.tile([128, 1], f32, tag=f"{pref}s2")
                nc.scalar.activation(scratch[0:rows, 0:width], src, ACTF.Square,
                                     accum_out=s2[0:rows])
                mean = pool.tile([128, 1], f32, tag=f"{pref}mean")
                nc.vector.tensor_scalar_mul(mean[0:rows], s1[0:rows], 1.0 / width)
                t2 = pool.tile([128, 1], f32, tag=f"{pref}t2")
                nc.vector.tensor_scalar_mul(t2[0:rows], s2[0:rows], 1.0 / width)
                vare = pool.tile([128, 1], f32, tag=f"{pref}vare")
                # mean^2 - s2/width = -(var); then *-1 + eps
                nc.vector.scalar_tensor_tensor(
                    out=vare[0:rows], in0=mean[0:rows], scalar=mean[0:rows],
                    in1=t2[0:rows], op0=OP.mult, op1=OP.subtract)
                nc.vector.tensor_scalar(vare[0:rows], vare[0:rows],
                                        -1.0, LN_EPS, op0=OP.mult, op1=OP.add)
                nc.vector.reciprocal(vare[0:rows], vare[0:rows])
                rstd = pool.tile([128, 1], f32, tag=f"{pref}rstd")
                nc.scalar.sqrt(rstd[0:rows], vare[0:rows])
                mrn = pool.tile([128, 1], f32, tag=f"{pref}mrn")
                nc.vector.tensor_mul(mrn[0:rows], mean[0:rows], rstd[0:rows])
                nc.vector.tensor_scalar_mul(mrn[0:rows], mrn[0:rows], -1.0)
                return rstd, mrn

            def ln_scales_dve(pool, pref, src, rows, width):  # noqa: same iface sans scratch
                bns = pool.tile([128, 4, 6], f32, tag=f"{pref}bns")
                wq4 = width // 4
                for j in range(4):
                    nc.vector.bn_stats(bns[0:rows, j, :], src[0:rows, wq4 * j:wq4 * (j + 1)])
                agg = pool.tile([128, 2], f32, tag=f"{pref}agg")
                nc.vector.bn_aggr(agg[0:rows], bns[0:rows])
                vare = pool.tile([128, 1], f32, tag=f"{pref}vare")
                nc.vector.tensor_scalar_add(vare[0:rows], agg[0:rows, 1:2], LN_EPS)
                nc.vector.reciprocal(vare[0:rows], vare[0:rows])
                rstd = pool.tile([128, 1], f32, tag=f"{pref}rstd")
                nc.scalar.sqrt(rstd[0:rows], vare[0:rows])
                mrn = pool.tile([128, 1], f32, tag=f"{pref}mrn")
                nc.vector.tensor_mul(mrn[0:rows], agg[0:rows, 0:1], rstd[0:rows])
                nc.vector.tensor_scalar_mul(mrn[0:rows], mrn[0:rows], -1.0)
                return rstd, mrn

            def p2_tile(t, on_act=False):
                rows = 128 if t < 12 else 64
                zt = p2.tile([128, N], f32, tag="zt", bufs=1)
                base = t * 128 * N
                nc.sync.dma_start(
                    zt[0:rows, :],
                    z_d.ap()[base:base + rows * N].rearrange("(p f) -> p f", p=rows))
                ot = p2.tile([128, N], f32, tag="ot")
                if on_act:
                    rstd, mru = ln_scales(p2, "a", zt[0:rows], rows, N, ot)
                else:
                    rstd, mru = ln_scales_dve(p2, "a", zt[0:rows], rows, N)
                nc.scalar.activation(ot[0:rows], zt[0:rows], ACTF.Identity,
                                     bias=mru[0:rows], scale=rstd[0:rows])
                otb = []
                for kc in range(16):
                    pot = pps.tile([128, 128], f32, tag="tp")
                    nc.tensor.transpose(pot[:, 0:rows], ot[0:rows, 128 * kc:128 * (kc + 1)],
                                        ident[0:rows, 0:rows])
                    ob = p2.tile([128, 128], bf16, tag=f"ob{kc}", bufs=2)
                    nc.scalar.copy(ob[:, 0:rows], pot[:, 0:rows])
                    otb.append(ob)
                # xd = o @ Wd ; u = xd + o ; LN2 ; x2 -> DRAM (in-place in ut)
                ut = p2.tile([128, N], f32, tag="ut")
                for jt in range(4):
                    px = pps.tile([128, 512], f32, tag="px")
                    for kc in range(16):
                        nc.tensor.matmul(px[0:rows], otb[kc][:, 0:rows],
                                         wdb[kc][:, 512 * jt:512 * (jt + 1)],
                                         start=(kc == 0), stop=(kc == 15))
                    nc.vector.tensor_add(ut[0:rows, 512 * jt:512 * (jt + 1)],
                                         px[0:rows], ot[0:rows, 512 * jt:512 * (jt + 1)])
                if on_act:
                    rstd2, mru2 = ln_scales(p2, "b", ut[0:rows], rows, N, ot)
                else:
                    rstd2, mru2 = ln_scales_dve(p2, "b", ut[0:rows], rows, N)
                nc.scalar.activation(ut[0:rows], ut[0:rows], ACTF.Identity,
                                     bias=mru2[0:rows], scale=rstd2[0:rows])
                base2 = t * 128 * N
                nc.sync.dma_start(
                    x2s_d.ap()[base2:base2 + rows * N].rearrange("(p f) -> p f", p=rows),
                    ut[0:rows])

            # R1: Wd prefetch + P1 blocks 0-4
            for kc in range(16):
                wtmp = p2.tile([128, N], f32, tag="ot")
                nc.sync.dma_start(wtmp, wd_d[128 * kc:128 * (kc + 1), :])
                wb = wdp.tile([128, N], bf16, tag=f"wd{kc}")
                nc.scalar.copy(wb, wtmp)
                wdb.append(wb)
            live = {}
            prev = None
            for blk in range(5):
                tif = p1_stageA(blk)
                if prev is not None:
                    p1_stageB(prev, *live.pop(prev))
                live[blk] = p1_stageG(blk, tif)
                prev = blk
            p1_stageB(prev, *live.pop(prev))
            tc.strict_bb_all_engine_barrier()
            # R2: P1 blocks 5-7 interleaved with P2 tiles 0-6; emission order
            # keeps every engine FIFO free of far-dependency head-of-line
            # blockers while the DVE chews topk.
            prev = None
            t_iter = iter(range(7))

            def emit_tiles(k):
                for _ in range(k):
                    t = next(t_iter, None)
                    if t is not None:
                        p2_tile(t, on_act=True)
            for blk in range(5, NBLK):
                tif = p1_stageA(blk)
                emit_tiles(1)
                if prev is not None:
                    p1_stageB(prev, *live.pop(prev))
                live[blk] = p1_stageG(blk, tif)
                emit_tiles(1)
                prev = blk
            p1_stageB(prev, *live.pop(prev))
            emit_tiles(7)
            tc.strict_bb_all_engine_barrier()
            # R3: P2 tiles 7-12
            for t in range(7, 13):
                p2_tile(t)

        tc.strict_bb_all_engine_barrier()
        groups = [[0, 1], [2, 3], [4, 5], [6, 7]]
        xgv = xg_d.ap().rearrange("a (m n) -> (a m) n", n=N)
        if nocc:
            nc.gpsimd.dma_start(xg_d[0, :], x2s_d[:])
            nc.gpsimd.dma_start(xg_d[1, :], x2s_d[:])
            nc.gpsimd.dma_start(x2a_d[:, :], xgv[:, 0:NL])
        else:
            nc.gpsimd.collective_compute(
                "AllGather", mybir.AluOpType.bypass, replica_groups=groups,
                ins=[x2s_d.ap().opt()], outs=[xg_d.ap().opt()])
            rv = nc.gpsimd.cc_rank(groups)
            with tc.If(rv < 1) as cmp:
                nc.gpsimd.dma_start(x2a_d[:, :], xgv[:, 0:NL])
            with cmp.Else():
                nc.gpsimd.dma_start(x2a_d[:, :], xgv[:, NL:N])
        tc.strict_bb_all_engine_barrier()
        if debug:
            nc.sync.dma_start(dbg_z[:], z_d[:])
            nc.sync.dma_start(dbg_x2a[:, :], x2a_d[:, :])

        # ---------------- P3a/P3b pipeline ----------------
        with tc.tile_pool(name="p3", bufs=2) as p3, \
             tc.tile_pool(name="p3x5", bufs=1) as p3x5, \
             tc.tile_pool(name="p3ps", bufs=2, space="PSUM") as p3ps, \
             tc.tile_pool(name="p3b", bufs=2) as p3b, \
             tc.tile_pool(name="p3bps", bufs=2, space="PSUM") as p3bps:
            x5b = [p3x5.tile([128, KT * C], f32, tag=f"x5b{i}", name=f"x5b{i}")
                   for i in range(8)]

            def p3b_block(nsub):
                ansb = p3b.tile([128, K, C], f32, tag="ansb")
                ibase = 8 * K * nsub
                for j, (s0, nidx, reg) in enumerate(
                        [(0, 1024, r1k), (8, 1024, r1k), (16, 512, r512)]):
                    nc.gpsimd.dma_gather(
                        ansb[:, s0:s0 + nidx // 128, :],
                        x5_d[nsub * 6400:(nsub + 1) * 6400, :],
                        widx5[:, ibase + 64 * j:ibase + 64 * j + nidx // 16].bitcast(i16),
                        nidx, reg, C)
                for t5 in range(10):
                    pta = p3bps.tile([128, 128], f32, tag="pta")
                    nc.tensor.transpose(
                        pta, ansb.rearrange("p a b -> p (a b)")[:, 128 * t5:128 * (t5 + 1)],
                        ident)
                    outc = p3b.tile([128, 128], f32, tag="outc")
                    nc.vector.tensor_copy(outc, pta)
                    dst = AP(out_d, 2 * t5 * NL + 128 * nsub,
                             [[NL, 2], [K * NL, C], [1, 128]])
                    nc.sync.dma_start(dst, outc)

            for nt2 in range(2):
                for kk2 in range(25):
                    rsl = slice(128 * kk2, 128 * (kk2 + 1))
                    csl = slice(512 * nt2, 512 * (nt2 + 1))
                    r0 = p3.tile([128, 512], f32, tag="r0")
                    nc.sync.dma_start(r0, x2a_d[rsl, csl])
                    rw = p3.tile([128, 512], f32, tag="rw")
                    nc.sync.dma_start(rw, xwt_d[rsl, csl])
                    rr = p3.tile([128, 512], f32r, tag="rr")
                    nc.vector.tensor_add(rr, r0, rw)
                    px5 = p3ps.tile([128, 512], f32, tag="px5")
                    nc.tensor.matmul(px5, bdr, rr, start=True, stop=True)
                    x5p = p3.tile([128, 512], f32, tag="x5p")
                    nc.vector.tensor_copy(x5p, px5)
                    for c4 in range(4):
                        nsub = nt2 * 4 + c4
                        pt5 = p3ps.tile([128, 128], f32, tag="pt5")
                        nc.tensor.transpose(pt5, x5p[:, 128 * c4:128 * (c4 + 1)], ident)
                        if c4 % 2 == 0:
                            nc.scalar.copy(x5b[nsub][:, rsl], pt5)
                        else:
                            nc.vector.tensor_copy(x5b[nsub][:, rsl], pt5)
                for c4 in range(4):
                    nsub = nt2 * 4 + c4
                    nc.sync.dma_start(
                        x5_d.ap().rearrange("a b -> (a b)")
                        [nsub * 409600:(nsub + 1) * 409600]
                        .rearrange("(p f) -> p f", p=128), x5b[nsub])
                tc.strict_bb_all_engine_barrier()
                for c4 in range(4):
                    p3b_block(nt2 * 4 + c4)
        persist.__exit__(None, None, None)

    return nc


_NC_CACHE = {}


def _get_nc(debug=False):
    if debug not in _NC_CACHE:
        nc = build_nc(debug=debug)
        nc.compile()
        _NC_CACHE[debug] = nc
    return _NC_CACHE[debug]


def make_in_maps(inputs):
    x = np.asarray(inputs["x"], np.float32)
    ident = np.eye(128, dtype=np.float32)
    pcol = (50.0 * np.arange(128, dtype=np.float32)).reshape(128, 1)
    shuf = np.zeros((128, 1024), np.float32)
    for b in range(8):
        for pp in range(128):
            shuf[16 * b + pp % 16, 128 * b + pp] = 1.0
    w = {k: np.ascontiguousarray(np.asarray(inputs[k], np.float32))
         for k in ["Wq", "Wk", "Wv", "Wd", "Wfc"]}
    for k in ["bq", "bk", "bv", "bd", "bfc", "beta1", "beta2"]:
        assert not np.any(np.asarray(inputs[k])), f"{k} must be zero"
    for k in ["g1", "g2"]:
        assert np.all(np.asarray(inputs[k]) == 1.0), f"{k} must be ones"
    in_maps = []
    for core in range(8):
        b, h = core // 2, core % 2
        in_maps.append(dict(
            xb=np.ascontiguousarray(x[b]),
            xq=np.ascontiguousarray(x[b][:, h * NL:(h + 1) * NL]),
            wq=w["Wq"], wk=w["Wk"], wv=w["Wv"], wd=w["Wd"], wfc=w["Wfc"],
            ident=ident, pcol=pcol, shuf=shuf))
    return in_maps


def kernel(**inputs):
    from concourse.bass_utils import run_bass_kernel_spmd
    nc = _get_nc(debug=False)
    in_maps = make_in_maps(inputs)
    res = run_bass_kernel_spmd(nc, in_maps, core_ids=list(range(8)))
    out = np.zeros((B, C, K, N), np.float32)
    for core in range(8):
        b, h = core // 2, core % 2
        out[b, :, :, h * NL:(h + 1) * NL] = res.results[core]["outp"]
    return out


# revision 27
# speedup vs baseline: 1.0605x; 1.0605x over previous
"""Trainium2 Bass kernel for nn_AttnKnn (retrieval KNN attention), 8-core SPMD.

Sharding: core = (batch b, n-half h); each core runs the full pipeline for its
1024 query points. Only cross-core exchange: pairwise AllGather of x2 column
halves (dense stage is m-row-split; fc stage needs n-column-split).

Key structure vs the naive version:
- Neighbor gather uses batched SWDGE dma_gather (1024 rows per instruction,
  ring-limited) instead of per-k indirect DMAs: kvx rows are [V | x] (512B).
- K projection is folded into the query side: logits = sum_c xwn_c * tq_c
  with tq = (x^T Wq) Wk^T, so K rows are never gathered.
- The wrapped int16 index layout dma_gather needs ([16, n/16] replicated) is
  built on-chip with 8 partition-shuffle matmuls against a constant matrix.
- Software pipeline: the PE-bound dense stage (P2) overlaps the DVE-bound
  KNN stage (P1); the fc stage (P3a) overlaps the final gather (P3b).

Assumes (asserted on host): g1 == g2 == 1, beta1 == beta2 == 0, biases == 0
(guaranteed by the problem spec fills).
"""
import numpy as np

B, C, N, Q = 4, 64, 2048, 64
KT, K = 50, 20
NL = N // 2
ML = NL * KT * C // N          # 1600
NBLK = NL // 128               # 8
SCALE = float(1.0 / np.sqrt(np.float32(N)))
LN_EPS = 1e-5
NEG = -1e30


def build_nc(debug=False, nocc=False):
    import concourse.bass as bass
    import concourse.bacc as bacc
    import concourse.mybir as mybir
    from concourse.tile import TileContext
    from concourse import library_config

    f32 = mybir.dt.float32
    f32r = mybir.dt.float32r
    bf16 = mybir.dt.bfloat16
    i16 = mybir.dt.int16
    u16 = mybir.dt.uint16
    ACTF = mybir.ActivationFunctionType
    OP = mybir.AluOpType
    AX = mybir.AxisListType
    AP = bass.AP

    def bcast_inner(t, inner):
        return AP(t.tensor, t.offset, [list(d) for d in t.ap] + [[0, inner]])

    nc = bacc.Bacc("TRN2", target_bir_lowering=False, num_devices=8)

    xb_d = nc.dram_tensor("xb", [C, N], f32, kind="ExternalInput")
    xq_d = nc.dram_tensor("xq", [C, NL], f32, kind="ExternalInput")
    wq_d = nc.dram_tensor("wq", [C, Q], f32, kind="ExternalInput")
    wk_d = nc.dram_tensor("wk", [C, Q], f32, kind="ExternalInput")
    wv_d = nc.dram_tensor("wv", [C, C], f32, kind="ExternalInput")
    wd_d = nc.dram_tensor("wd", [N, N], f32, kind="ExternalInput")
    wfc_d = nc.dram_tensor("wfc", [C, C], f32, kind="ExternalInput")
    ident_d = nc.dram_tensor("ident", [128, 128], f32, kind="ExternalInput")
    pcol_d = nc.dram_tensor("pcol", [128, 1], f32, kind="ExternalInput")  # 50*p
    shuf_d = nc.dram_tensor("shuf", [128, 1024], f32, kind="ExternalInput")
    out_d = nc.dram_tensor("outp", [C, K, NL], f32, kind="ExternalOutput")

    kvx_d = nc.dram_tensor("kvx", [N, 128], f32)
    z_d = nc.dram_tensor("zs", [NL * KT * C], f32)
    xwt_d = nc.dram_tensor("xwt", [KT * C, NL], f32)
    xsw_d = nc.dram_tensor("xsw", [2, ML, NL], f32)
    x2a_d = nc.dram_tensor("x2a", [2 * ML, NL], f32)
    x5_d = nc.dram_tensor("x5d", [NL * KT, C], f32)
    if debug:
        dbg_idx = nc.dram_tensor("dbg_idx", [NL, KT], u16, kind="ExternalOutput")
        dbg_widx = nc.dram_tensor("dbg_widx", [128, 400], u16, kind="ExternalOutput")
        dbg_z = nc.dram_tensor("dbg_z", [NL * KT * C], f32, kind="ExternalOutput")
        dbg_x2a = nc.dram_tensor("dbg_x2a", [2 * ML, NL], f32, kind="ExternalOutput")

    with TileContext(nc) as tc:
        nc.gpsimd.load_library(library_config.mlp)
        r1k = nc.gpsimd.to_reg(1024)
        r512 = nc.gpsimd.to_reg(512)
        r256 = nc.gpsimd.to_reg(256)
        persist = tc.tile_pool(name="persist", bufs=1)
        pp = persist.__enter__()
        ident = pp.tile([128, 128], f32)
        nc.sync.dma_start(ident, ident_d[:, :])
        pcol = pp.tile([128, 1], f32)
        nc.sync.dma_start(pcol, pcol_d[:, :])
        shufs = pp.tile([128, 1024], f32)
        nc.sync.dma_start(shufs, shuf_d[:, :])
        wfcs = pp.tile([C, C], f32)
        nc.sync.dma_start(wfcs, wfc_d[:, :])
        bd0 = pp.tile([128, 128], f32)
        nc.vector.memset(bd0, 0.0)
        nc.vector.tensor_copy(bd0[0:64, 0:64], wfcs)
        nc.vector.tensor_copy(bd0[64:128, 64:128], wfcs)
        bdr = pp.tile([128, 128], f32r)
        nc.vector.tensor_copy(bdr, bd0)
        msqn = pp.tile([128, NBLK], f32)
        tqa = pp.tile([128, NBLK * C], f32)
        xbar = pp.tile([65, N], f32)
        xq2r = pp.tile([65, NL], f32)
        widx5 = pp.tile([128, NBLK * 8 * K], u16)   # wrapped fc-gather indices

        # ---------------- P0 ----------------
        with tc.tile_pool(name="p0", bufs=2) as p0, \
             tc.tile_pool(name="p0ps", bufs=2, space="PSUM") as p0ps:
            xbs = p0.tile([C, N], f32, bufs=1)
            nc.sync.dma_start(xbs, xb_d[:, :])
            xqs = p0.tile([C, NL], f32, bufs=1)
            nc.sync.dma_start(xqs, xq_d[:, :])
            wqs = p0.tile([C, Q], f32, bufs=1)
            nc.sync.dma_start(wqs, wq_d[:, :])
            wks = p0.tile([C, Q], f32, bufs=1)
            nc.sync.dma_start(wks, wk_d[:, :])
            wvs = p0.tile([C, C], f32, bufs=1)
            nc.sync.dma_start(wvs, wv_d[:, :])
            ones = p0.tile([C, 1], f32, bufs=1)
            nc.vector.memset(ones, 1.0)
            Ms = p0.tile([C, C], f32, bufs=1)
            sqrow = p0.tile([1, N], f32, bufs=1)
            sqqrow = p0.tile([1, NL], f32, bufs=1)
            xba = p0.tile([65, N], f32, bufs=1)
            xq2 = p0.tile([65, NL], f32, bufs=1)
            # M = Wq @ Wk^T  (for logits = sum_c xwn_c * (q @ Wk^T)_c)
            pwq = p0ps.tile([Q, C], f32, tag="ps0")
            nc.tensor.transpose(pwq, wqs, ident[0:C, 0:C])
            wqT = p0.tile([Q, C], f32, bufs=1)
            nc.scalar.copy(wqT, pwq)
            pwk = p0ps.tile([Q, C], f32, tag="ps0")
            nc.tensor.transpose(pwk, wks, ident[0:C, 0:C])
            wkT = p0.tile([Q, C], f32, bufs=1)
            nc.scalar.copy(wkT, pwk)
            pM = p0ps.tile([C, C], f32, tag="ps0")
            nc.tensor.matmul(pM, wqT, wkT, start=True, stop=True)
            nc.scalar.copy(Ms, pM)

            xsq = p0.tile([C, N], f32, bufs=1)
            nc.scalar.square(xsq, xbs)
            for j in range(4):
                pq = p0ps.tile([1, 512], f32, tag="ps0")
                nc.tensor.matmul(pq, ones, xsq[:, 512 * j:512 * (j + 1)],
                                 start=True, stop=True)
                nc.scalar.copy(sqrow[:, 512 * j:512 * (j + 1)], pq)
            xsqq = p0.tile([C, NL], f32, bufs=1)
            nc.scalar.square(xsqq, xqs)
            for j in range(2):
                pq2 = p0ps.tile([1, 512], f32, tag="ps0")
                nc.tensor.matmul(pq2, ones, xsqq[:, 512 * j:512 * (j + 1)],
                                 start=True, stop=True)
                nc.scalar.copy(sqqrow[:, 512 * j:512 * (j + 1)], pq2)
            # kvx rows: [V | x^T]
            for j in range(16):
                sl = slice(128 * j, 128 * (j + 1))
                pt = p0ps.tile([128, C], f32, tag="ps0")
                nc.tensor.transpose(pt, xbs[:, sl], ident[0:C, 0:C])
                xtt = p0.tile([128, C], f32)
                nc.scalar.copy(xtt, pt)
                nc.sync.dma_start(kvx_d[sl, 64:128], xtt)
                pv = p0ps.tile([128, C], f32, tag="ps0")
                nc.tensor.matmul(pv, xbs[:, sl], wvs, start=True, stop=True)
                vat = p0.tile([128, C], f32)
                nc.scalar.copy(vat, pv)
                nc.sync.dma_start(kvx_d[sl, 0:64], vat)
            nc.vector.tensor_copy(xba[0:C, :], xbs)
            nc.scalar.activation(xba[C:C + 1, :], sqrow, ACTF.Identity,
                                 bias=0.0, scale=-1.0)
            nc.scalar.activation(xq2[0:C, :], xqs, ACTF.Identity,
                                 bias=0.0, scale=2.0)
            nc.vector.memset(xq2[C:C + 1, :], 1.0)
            nc.vector.tensor_copy(xbar, xba)
            nc.vector.tensor_copy(xq2r, xq2)
            for blk in range(NBLK):
                sl = slice(128 * blk, 128 * (blk + 1))
                ptq = p0ps.tile([128, C], f32, tag="ps0")
                nc.tensor.matmul(ptq, xqs[:, sl], Ms, start=True, stop=True)
                nc.scalar.copy(tqa[:, C * blk:C * (blk + 1)], ptq)
                psn = p0ps.tile([128, 1], f32, tag="ps0")
                nc.tensor.transpose(psn, sqqrow[:, sl], ident[0:1, 0:1])
                nc.scalar.activation(msqn[:, blk:blk + 1], psn, ACTF.Identity,
                                     bias=0.0, scale=-1.0)

        tc.strict_bb_all_engine_barrier()

        # ---------------- P1 + P2 software pipeline ----------------
        with tc.tile_pool(name="p1", bufs=2) as p1, \
             tc.tile_pool(name="pps", bufs=2, space="PSUM") as pps, \
             tc.tile_pool(name="wd", bufs=1) as wdp, \
             tc.tile_pool(name="p2", bufs=2) as p2:

            def p1_stageA(blk):
                nsl = slice(128 * blk, 128 * (blk + 1))
                dist = p1.tile([128, N], f32, tag="dist")
                for j in range(4):
                    pd = pps.tile([128, 512], f32, tag="pd", bufs=1)
                    nc.tensor.matmul(pd, xq2r[:, nsl], xbar[:, 512 * j:512 * (j + 1)],
                                     start=True, stop=True)
                    nc.scalar.activation(dist[:, 512 * j:512 * (j + 1)], pd,
                                         ACTF.Identity, bias=msqn[:, blk:blk + 1],
                                         scale=1.0)
                ti = p1.tile([128, 56], u16, tag="ti")
                m8 = p1.tile([128, 8], f32, tag="m8")
                for it in range(7):
                    nc.vector.max(out=m8, in_=dist)
                    nc.vector.max_index(out=ti[:, 8 * it:8 * (it + 1)], in_max=m8,
                                        in_values=dist)
                    if it < 6:
                        nc.vector.match_replace(out=dist, in_to_replace=m8,
                                                in_values=dist, imm_value=NEG)
                if debug:
                    nc.sync.dma_start(dbg_idx[nsl, :], ti[:, 0:KT])
                tif = p1.tile([128, KT], f32, tag="tif")
                nc.vector.tensor_copy(tif, ti[:, 0:KT])
                return tif

            def p1_stageG(blk, tif):
                # wrapped idx layout: widx[p, 8i+b] = ti[16b+p%16, i]
                widx = p1.tile([128, 8 * KT], u16, tag="widx")
                for b in range(8):
                    psh = pps.tile([128, KT], f32, tag="sh", bufs=1)
                    nc.tensor.matmul(psh, shufs[:, 128 * b:128 * (b + 1)], tif,
                                     start=True, stop=True)
                    nc.scalar.copy(
                        AP(widx.tensor, widx.offset + b,
                           [list(widx.ap[0]), [8, KT]]), psh)
                if debug:
                    nc.sync.dma_start(dbg_widx[:, :], widx)
                # batched neighbor gather: rows [V | x], 512B each
                gbuf = p1.tile([128, KT, 128], f32, tag="gbuf", bufs=1)
                for j in range(7):
                    nidx = 1024 if j < 6 else 256
                    ks = slice(8 * j, 8 * j + nidx // 128)
                    idx = widx[:, 64 * j:64 * j + nidx // 16].bitcast(i16)
                    reg = r1k if nidx == 1024 else r256
                    nc.gpsimd.dma_gather(gbuf[:, ks, :], kvx_d[:, :], idx,
                                         nidx, reg, 128)
                return (gbuf,)

            def p1_stageB(blk, gbuf):
                nsl = slice(128 * blk, 128 * (blk + 1))
                vbuf = gbuf[:, :, 0:64]
                xbuf = gbuf[:, :, 64:128]
                # xwn^T chunks -> DRAM (transpose straight out of gbuf);
                # emitted first so these Act copies drain before the softmax.
                # BIR allows only one free dim on transpose inputs, so
                # transpose per-k [128,64] slices and pair-pack the results.
                for c2 in range(25):
                    xwtt = p1.tile([128, 128], f32, tag="xwtt")
                    for h2 in range(2):
                        pxw = pps.tile([64, 128], f32, tag="pxw")
                        nc.tensor.transpose(pxw, gbuf[:, 2 * c2 + h2, 64:128],
                                            ident)
                        nc.scalar.copy(xwtt[64 * h2:64 * (h2 + 1), :], pxw)
                    nc.sync.dma_start(xwt_d[128 * c2:128 * (c2 + 1), nsl], xwtt)
                # logits = sum_c x_c * tq_c
                qsl = tqa[:, C * blk:C * (blk + 1)]
                prod = p1.tile([128, KT, Q], f32, tag="pz")
                nc.vector.tensor_mul(prod, xbuf,
                                     AP(qsl.tensor, qsl.offset,
                                        [list(qsl.ap[0]), [0, KT], list(qsl.ap[1])]))
                logits = p1.tile([128, KT], f32, tag="logits")
                nc.vector.tensor_reduce(out=logits, in_=prod, op=OP.add, axis=AX.X)
                lm8 = p1.tile([128, 8], f32, tag="lm8")
                nc.vector.max(out=lm8, in_=logits)
                mneg = p1.tile([128, 1], f32, tag="mneg")
                nc.vector.tensor_scalar_mul(mneg, lm8[:, 0:1], -SCALE)
                attnu = p1.tile([128, KT], f32, tag="attnu")
                nc.scalar.activation(attnu, logits, ACTF.Exp, bias=mneg, scale=SCALE)
                asum = p1.tile([128, 1], f32, tag="asum")
                nc.vector.tensor_reduce(out=asum, in_=attnu, op=OP.add, axis=AX.X)
                rsum = p1.tile([128, 1], f32, tag="rsum")
                nc.vector.reciprocal(rsum, asum)
                attn = p1.tile([128, KT], f32, tag="attn")
                nc.vector.tensor_scalar_mul(attn, attnu, rsum)
                # bottom-20 of logits (== argsort(attn)[:20])
                negl = p1.tile([128, KT], f32, tag="negl")
                nc.vector.tensor_scalar_mul(negl, logits, -1.0)
                kti = p1.tile([128, 24], u16, tag="kti")
                nm8 = p1.tile([128, 8], f32, tag="nm8")
                for it in range(3):
                    nc.vector.max(out=nm8, in_=negl)
                    nc.vector.max_index(out=kti[:, 8 * it:8 * (it + 1)], in_max=nm8,
                                        in_values=negl)
                    if it < 2:
                        nc.vector.match_replace(out=negl, in_to_replace=nm8,
                                                in_values=negl, imm_value=NEG)
                # z = attn*v + xwn -> DRAM
                z3 = p1.tile([128, KT, C], f32, tag="pz")
                nc.vector.tensor_mul(z3, vbuf, bcast_inner(attn, C))
                nc.vector.tensor_add(z3, z3, xbuf)
                zview = z_d.ap()[blk * 409600:(blk + 1) * 409600].rearrange(
                    "(p f) -> p f", p=128)
                nc.sync.dma_start(zview, z3)
                # fc-gather indices (x5 row = 50*q_local + k), wrapped per block
                kself = p1.tile([128, 24], f32, tag="kself")
                nc.vector.tensor_copy(kself, kti)
                kofff = p1.tile([128, K], f32, tag="kofff")
                nc.vector.tensor_add(kofff, kself[:, 0:K],
                                     AP(pcol.tensor, pcol.offset,
                                        [list(pcol.ap[0]), [0, K]]))
                for b in range(8):
                    psh5 = pps.tile([128, KT], f32, tag="sh", bufs=1)
                    nc.tensor.matmul(psh5[:, 0:K], shufs[:, 128 * b:128 * (b + 1)], kofff,
                                     start=True, stop=True)
                    nc.scalar.copy(
                        AP(widx5.tensor, widx5.offset + 8 * K * blk + b,
                           [list(widx5.ap[0]), [8, K]]), psh5[:, 0:K])

            wdb = []

            def ln_scales(pool, pref, src, rows, width, scratch):
                """mean/var over `width` via two Act accumulate passes writing
                throwaway data into `scratch` (a dead/overwritten-later tile);
                returns (rstd, mrn) with mrn = -mean*rstd, both [rows, 1]."""
                s1 = pool.tile([128, 1], f32, tag=f"{pref}s1")
                nc.scalar.activation(scratch[0:rows, 0:width], src, ACTF.Identity,
                                     accum_out=s1[0:rows])
                s2 = pool.tile([128, 1], f32, tag=f"{pref}s2")
                nc.scalar.activation(scratch[0:rows, 0:width], src, ACTF.Square,
                                     accum_out=s2[0:rows])
                mean = pool.tile([128, 1], f32, tag=f"{pref}mean")
                nc.vector.tensor_scalar_mul(mean[0:rows], s1[0:rows], 1.0 / width)
                t2 = pool.tile([128, 1], f32, tag=f"{pref}t2")
                nc.vector.tensor_scalar_mul(t2[0:rows], s2[0:rows], 1.0 / width)
                vare = pool.tile([128, 1], f32, tag=f"{pref}vare")
                # mean^2 - s2/width = -(var); then *-1 + eps
                nc.vector.scalar_tensor_tensor(
                    out=vare[0:rows], in0=mean[0:rows], scalar=mean[0:rows],
                    in1=t2[0:rows], op0=OP.mult, op1=OP.subtract)
                nc.vector.tensor_scalar(vare[0:rows], vare[0:rows],
                                        -1.0, LN_EPS, op0=OP.mult, op1=OP.add)
                nc.vector.reciprocal(vare[0:rows], vare[0:rows])
                rstd = pool.tile([128, 1], f32, tag=f"{pref}rstd")
                nc.scalar.sqrt(rstd[0:rows], vare[0:rows])
                mrn = pool.tile([128, 1], f32, tag=f"{pref}mrn")
                nc.vector.tensor_mul(mrn[0:rows], mean[0:rows], rstd[0:rows])
                nc.vector.tensor_scalar_mul(mrn[0:rows], mrn[0:rows], -1.0)
                return rstd, mrn

            def ln_scales_dve(pool, pref, src, rows, width):  # noqa: same iface sans scratch
                bns = pool.tile([128, 4, 6], f32, tag=f"{pref}bns")
                wq4 = width // 4
                for j in range(4):
                    nc.vector.bn_stats(bns[0:rows, j, :], src[0:rows, wq4 * j:wq4 * (j + 1)])
                agg = pool.tile([128, 2], f32, tag=f"{pref}agg")
                nc.vector.bn_aggr(agg[0:rows], bns[0:rows])
                vare = pool.tile([128, 1], f32, tag=f"{pref}vare")
                nc.vector.tensor_scalar_add(vare[0:rows], agg[0:rows, 1:2], LN_EPS)
                nc.vector.reciprocal(vare[0:rows], vare[0:rows])
                rstd = pool.tile([128, 1], f32, tag=f"{pref}rstd")
                nc.scalar.sqrt(rstd[0:rows], vare[0:rows])
                mrn = pool.tile([128, 1], f32, tag=f"{pref}mrn")
                nc.vector.tensor_mul(mrn[0:rows], agg[0:rows, 0:1], rstd[0:rows])
                nc.vector.tensor_scalar_mul(mrn[0:rows], mrn[0:rows], -1.0)
                return rstd, mrn

            def p2_tile(t, on_act=False):
                rows = 128 if t < 12 else 64
                zt = p2.tile([128, N], f32, tag="zt", bufs=1)
                base = t * 128 * N
                nc.sync.dma_start(
                    zt[0:rows, :],
                    z_d.ap()[base:base + rows * N].rearrange("(p f) -> p f", p=rows))
                ot = p2.tile([128, N], f32, tag="ot")
                if on_act:
                    rstd, mru = ln_scales(p2, "a", zt[0:rows], rows, N, ot)
                else:
                    rstd, mru = ln_scales_dve(p2, "a", zt[0:rows], rows, N)
                nc.scalar.activation(ot[0:rows], zt[0:rows], ACTF.Identity,
                                     bias=mru[0:rows], scale=rstd[0:rows])
                otb = []
                for kc in range(16):
                    pot = pps.tile([128, 128], f32, tag="tp")
                    nc.tensor.transpose(pot[:, 0:rows], ot[0:rows, 128 * kc:128 * (kc + 1)],
                                        ident[0:rows, 0:rows])
                    ob = p2.tile([128, 128], bf16, tag=f"ob{kc}", bufs=2)
                    nc.scalar.copy(ob[:, 0:rows], pot[:, 0:rows])
                    otb.append(ob)
                # xd = o @ Wd ; u = xd + o ; LN2 ; x2 -> DRAM (in-place in ut)
                ut = p2.tile([128, N], f32, tag="ut")
                for jt in range(4):
                    px = pps.tile([128, 512], f32, tag="px")
                    for kc in range(16):
                        nc.tensor.matmul(px[0:rows], otb[kc][:, 0:rows],
                                         wdb[kc][:, 512 * jt:512 * (jt + 1)],
                                         start=(kc == 0), stop=(kc == 15))
                    nc.vector.tensor_add(ut[0:rows, 512 * jt:512 * (jt + 1)],
                                         px[0:rows], ot[0:rows, 512 * jt:512 * (jt + 1)])
                if on_act:
                    rstd2, mru2 = ln_scales(p2, "b", ut[0:rows], rows, N, ot)
                else:
                    rstd2, mru2 = ln_scales_dve(p2, "b", ut[0:rows], rows, N)
                nc.scalar.activation(ut[0:rows], ut[0:rows], ACTF.Identity,
                                     bias=mru2[0:rows], scale=rstd2[0:rows])
                nc.sync.dma_start(xsw_d[0, 128 * t:128 * t + rows, :],
                                  ut[0:rows, 0:NL])
                nc.sync.dma_start(xsw_d[1, 128 * t:128 * t + rows, :],
                                  ut[0:rows, NL:N])

            # R1: Wd prefetch + P1 blocks 0-4
            for kc in range(16):
                wtmp = p2.tile([128, N], f32, tag="ot")
                nc.sync.dma_start(wtmp, wd_d[128 * kc:128 * (kc + 1), :])
                wb = wdp.tile([128, N], bf16, tag=f"wd{kc}")
                nc.scalar.copy(wb, wtmp)
                wdb.append(wb)
            live = {}
            prev = None
            for blk in range(4):
                tif = p1_stageA(blk)
                if prev is not None:
                    p1_stageB(prev, *live.pop(prev))
                live[blk] = p1_stageG(blk, tif)
                prev = blk
            p1_stageB(prev, *live.pop(prev))
            tc.strict_bb_all_engine_barrier()
            # R2: P1 blocks 5-7 interleaved with P2 tiles 0-6; emission order
            # keeps every engine FIFO free of far-dependency head-of-line
            # blockers while the DVE chews topk.
            prev = None
            t_iter = iter(range(6))

            def emit_tiles(k):
                for _ in range(k):
                    t = next(t_iter, None)
                    if t is not None:
                        p2_tile(t, on_act=True)
            for blk in range(4, NBLK):
                tif = p1_stageA(blk)
                emit_tiles(1)
                if prev is not None:
                    p1_stageB(prev, *live.pop(prev))
                live[blk] = p1_stageG(blk, tif)
                emit_tiles(1)
                prev = blk
            p1_stageB(prev, *live.pop(prev))
            emit_tiles(6)
            tc.strict_bb_all_engine_barrier()
            # R3: P2 tiles 6-12
            for t in range(6, 13):
                p2_tile(t)

        tc.strict_bb_all_engine_barrier()
        groups = [[0, 1], [2, 3], [4, 5], [6, 7]]
        if nocc:
            nc.gpsimd.dma_start(
                x2a_d.ap().rearrange("a b -> (a b)"),
                xsw_d.ap().rearrange("a b c -> (a b c)"))
        else:
            # AllToAll over the pair: rank h's section j goes to rank j; the
            # concatenated output [rows R0 | rows R1] of column-half h IS x2a.
            nc.gpsimd.collective_compute(
                "AllToAll", mybir.AluOpType.bypass, replica_groups=groups,
                ins=[xsw_d.ap().opt()], outs=[x2a_d.ap().opt()])
        tc.strict_bb_all_engine_barrier()
        if debug:
            nc.sync.dma_start(dbg_z[:], z_d[:])
            nc.sync.dma_start(dbg_x2a[:, :], x2a_d[:, :])

        # ---------------- P3a/P3b pipeline ----------------
        with tc.tile_pool(name="p3", bufs=2) as p3, \
             tc.tile_pool(name="p3x5", bufs=1) as p3x5, \
             tc.tile_pool(name="p3ps", bufs=2, space="PSUM") as p3ps, \
             tc.tile_pool(name="p3b", bufs=2) as p3b, \
             tc.tile_pool(name="p3bps", bufs=2, space="PSUM") as p3bps:
            x5b = [p3x5.tile([128, KT * C], f32, tag=f"x5b{i}", name=f"x5b{i}")
                   for i in range(8)]

            def p3b_block(nsub):
                ansb = p3b.tile([128, K, C], f32, tag="ansb")
                ibase = 8 * K * nsub
                for j, (s0, nidx, reg) in enumerate(
                        [(0, 1024, r1k), (8, 1024, r1k), (16, 512, r512)]):
                    nc.gpsimd.dma_gather(
                        ansb[:, s0:s0 + nidx // 128, :],
                        x5_d[nsub * 6400:(nsub + 1) * 6400, :],
                        widx5[:, ibase + 64 * j:ibase + 64 * j + nidx // 16].bitcast(i16),
                        nidx, reg, C)
                for t5 in range(10):
                    pta = p3bps.tile([128, 128], f32, tag="pta")
                    nc.tensor.transpose(
                        pta, ansb.rearrange("p a b -> p (a b)")[:, 128 * t5:128 * (t5 + 1)],
                        ident)
                    outc = p3b.tile([128, 128], f32, tag="outc")
                    nc.vector.tensor_copy(outc, pta)
                    dst = AP(out_d, 2 * t5 * NL + 128 * nsub,
                             [[NL, 2], [K * NL, C], [1, 128]])
                    nc.sync.dma_start(dst, outc)

            for nt2 in range(2):
                for kk2 in range(25):
                    rsl = slice(128 * kk2, 128 * (kk2 + 1))
                    csl = slice(512 * nt2, 512 * (nt2 + 1))
                    r0 = p3.tile([128, 512], f32, tag="r0")
                    nc.sync.dma_start(r0, x2a_d[rsl, csl])
                    rw = p3.tile([128, 512], f32, tag="rw")
                    nc.sync.dma_start(rw, xwt_d[rsl, csl])
                    rr = p3.tile([128, 512], f32r, tag="rr")
                    nc.vector.tensor_add(rr, r0, rw)
                    px5 = p3ps.tile([128, 512], f32, tag="px5")
                    nc.tensor.matmul(px5, bdr, rr, start=True, stop=True)
                    x5p = p3.tile([128, 512], f32, tag="x5p")
                    nc.scalar.copy(x5p, px5)
                    for c4 in range(4):
                        nsub = nt2 * 4 + c4
                        pt5 = p3ps.tile([128, 128], f32, tag="pt5")
                        nc.tensor.transpose(pt5, x5p[:, 128 * c4:128 * (c4 + 1)], ident)
                        if c4 % 2 == 0:
                            nc.scalar.copy(x5b[nsub][:, rsl], pt5)
                        else:
                            nc.vector.tensor_copy(x5b[nsub][:, rsl], pt5)
                for c4 in range(4):
                    nsub = nt2 * 4 + c4
                    nc.sync.dma_start(
                        x5_d.ap().rearrange("a b -> (a b)")
                        [nsub * 409600:(nsub + 1) * 409600]
                        .rearrange("(p f) -> p f", p=128), x5b[nsub])
                tc.strict_bb_all_engine_barrier()
                for c4 in range(4):
                    p3b_block(nt2 * 4 + c4)
        persist.__exit__(None, None, None)

    return nc


_NC_CACHE = {}


def _get_nc(debug=False):
    if debug not in _NC_CACHE:
        nc = build_nc(debug=debug)
        nc.compile()
        _NC_CACHE[debug] = nc
    return _NC_CACHE[debug]


def make_in_maps(inputs):
    x = np.asarray(inputs["x"], np.float32)
    ident = np.eye(128, dtype=np.float32)
    pcol = (50.0 * np.arange(128, dtype=np.float32)).reshape(128, 1)
    shuf = np.zeros((128, 1024), np.float32)
    for b in range(8):
        for pp in range(128):
            shuf[16 * b + pp % 16, 128 * b + pp] = 1.0
    w = {k: np.ascontiguousarray(np.asarray(inputs[k], np.float32))
         for k in ["Wq", "Wk", "Wv", "Wd", "Wfc"]}
    for k in ["bq", "bk", "bv", "bd", "bfc", "beta1", "beta2"]:
        assert not np.any(np.asarray(inputs[k])), f"{k} must be zero"
    for k in ["g1", "g2"]:
        assert np.all(np.asarray(inputs[k]) == 1.0), f"{k} must be ones"
    in_maps = []
    for core in range(8):
        b, h = core // 2, core % 2
        in_maps.append(dict(
            xb=np.ascontiguousarray(x[b]),
            xq=np.ascontiguousarray(x[b][:, h * NL:(h + 1) * NL]),
            wq=w["Wq"], wk=w["Wk"], wv=w["Wv"], wd=w["Wd"], wfc=w["Wfc"],
            ident=ident, pcol=pcol, shuf=shuf))
    return in_maps


def kernel(**inputs):
    from concourse.bass_utils import run_bass_kernel_spmd
    nc = _get_nc(debug=False)
    in_maps = make_in_maps(inputs)
    res = run_bass_kernel_spmd(nc, in_maps, core_ids=list(range(8)))
    out = np.zeros((B, C, K, N), np.float32)
    for core in range(8):
        b, h = core // 2, core % 2
        out[b, :, :, h * NL:(h + 1) * NL] = res.results[core]["outp"]
    return out
